# revision 14
# baseline (speedup 1.0000x reference)
"""Trainium2 Bass kernel for nn_Attention_34033320854122.

Dense transformer attention block: QKV proj -> causal depthwise conv+SiLU ->
per-head RMSNorm -> partial RoPE -> causal attention -> output projection.

Sharding: tensor-parallel over the 16 heads across 8 NeuronCores (2 heads =
256 channels per core). Each core computes q/k/v for its channels (full
contraction over D), runs attention for its 2 heads, and produces a partial
output projection (outT_partial = Wo[:, cols] @ attn_cols^T, bf16). The host
sums the 8 partials and transposes.

Numerics / fidelity notes (same conventions as the validated baseline):
- The reference negates the rotated RoPE sub-dim of BOTH q and k; the
  negation cancels in q . k and is skipped.
- softmax runs without max-subtraction (scores bounded well inside fp32 exp
  range for RMS-normed q/k at scale 1/sqrt(128)).
- Matmuls: QKV/Wo projections and attention PV in bf16; QK in float32r.
- The depthwise conv runs on the PE as 4 accumulating matmuls with
  diagonal stationary matrices diag(w[:, j]) built on the host.
- Activation functions are grouped by ACT table set (silu wave, then
  sqrt/exp interleaved per q-tile) to avoid table-load thrash.
- All DRAM tensors are packed host-side so each DMA is one contiguous run
  per partition (HWDGE issue time scales with descriptor count).
- The rms/rope chain for slice t+1 is emitted interleaved into the output
  projection of q-tile t so its cross-engine latency hides under PE work.
"""

import ml_dtypes
import numpy as np

import concourse.bacc as bacc
import concourse.tile as tile
import concourse.mybir as mybir
from concourse import bass_utils
from concourse.masks import make_identity

# Problem shape (hardcoded per contract)
B, T, D = 1, 2048, 2048
H, HD = 16, 128
RD = 64
KCONV = 4
EPS = 1e-5
NCORES = 8
CPC = D // NCORES      # channels per core = 256
MPC = CPC // HD        # head tiles per core = 2
NT = 512               # free-dim tile for matmuls
NQ = T // NT           # 4 q tiles
NKC = T // HD          # 16 key chunks of 128
KD = D // 128          # 16 contraction chunks
PAD = KCONV - 1        # causal conv history

F32 = mybir.dt.float32
F32R = mybir.dt.float32r
BF16 = mybir.dt.bfloat16

_COMPILED = None


def _build():
    nc = bacc.Bacc("TRN2", target_bir_lowering=False, debug=False,
                   num_devices=NCORES)

    d = {}
    # packed layouts: leading dim 128 = SBUF partition
    d["xP"] = nc.dram_tensor("xP", (128, NQ, KD, NT), BF16,
                             kind="ExternalInput").ap()
    d["wqP"] = nc.dram_tensor("wqP", (128, KD, CPC), BF16,
                              kind="ExternalInput").ap()
    d["wkP"] = nc.dram_tensor("wkP", (128, KD, CPC), BF16,
                              kind="ExternalInput").ap()
    d["wvP"] = nc.dram_tensor("wvP", (128, KD, CPC), BF16,
                              kind="ExternalInput").ap()
    d["woP"] = nc.dram_tensor("woP", (128, MPC, D), BF16,
                              kind="ExternalInput").ap()
    # trig: rows 0:64 = cos^T, rows 64:128 = sign-folded sin^T
    d["trig"] = nc.dram_tensor("trig", (128, T), F32, kind="ExternalInput").ap()
    # conv weights as diagonal stationaries [128, (pi,m,j)=24, 128]
    d["convd"] = nc.dram_tensor("convd", (128, 3 * MPC * KCONV, 128), BF16,
                                kind="ExternalInput").ap()
    # per-head norm weights [128, 2] (q, k)
    d["normw"] = nc.dram_tensor("normw", (128, 2), F32, kind="ExternalInput").ap()
    # causal mask strip [128, 896]: mask[kl, c] = 1.0 if kl <= c - 384
    d["maskb"] = nc.dram_tensor("maskb", (128, 896), BF16,
                                kind="ExternalInput").ap()
    # packed output: [p, tq, blk4, i, t'] -> out row (blk4*4+i)*128+p
    outP = nc.dram_tensor("outP", (128, NQ, 4, 4, NT), BF16,
                          kind="ExternalOutput").ap()

    inv_sqrt_hd = 1.0 / np.sqrt(HD)

    with tile.TileContext(nc) as tc:
        with (
            tc.tile_pool(name="consts", bufs=1) as consts,
            tc.tile_pool(name="raw", bufs=1) as rawp,
            tc.tile_pool(name="wqkv", bufs=1) as wqkvp,
            tc.tile_pool(name="wor", bufs=1) as worp,
            tc.tile_pool(name="final", bufs=1) as finalp,
            tc.tile_pool(name="xblk", bufs=2) as xp,
            tc.tile_pool(name="vsil", bufs=8) as vsilp,
            tc.tile_pool(name="scratch", bufs=2) as scr,
            tc.tile_pool(name="qn", bufs=2) as qnp,
            tc.tile_pool(name="exp", bufs=3) as expp,
            tc.tile_pool(name="attn", bufs=3) as attnp,
            tc.tile_pool(name="ostage", bufs=2) as ostp,
            tc.tile_pool(name="psa", bufs=3, space="PSUM") as psa,
            tc.tile_pool(name="psb", bufs=4, space="PSUM") as psb,
            tc.tile_pool(name="psone", bufs=1, space="PSUM") as psone,
        ):
            # ---- persistent buffers ----
            rawq = rawp.tile([128, MPC, T + PAD], BF16)
            rawk = rawp.tile([128, MPC, T + PAD], BF16)
            rawv = rawp.tile([128, MPC, T + PAD], BF16)
            qfT = finalp.tile([128, MPC, T], F32R)
            kfT = finalp.tile([128, MPC, T], F32R)
            vtr = finalp.tile([128, MPC, NKC, HD], BF16)
            w_all = wqkvp.tile([128, 3, KD, CPC], BF16)
            raws = (rawq, rawk, rawv)
            groups = [[(0, 0), (0, 1), (1, 0)], [(1, 1), (2, 0), (2, 1)]]

            # ---- startup DMAs: x tile 0 + weight k-blocks first ----
            xb0 = xp.tile([128, KD, NT], BF16, name="xb")
            nc.sync.dma_start(xb0[:, 0:4, :], d["xP"][:, 0, 0:4, :])
            wds = (d["wqP"], d["wkP"], d["wvP"])
            for kq in range(4):
                for pi in range(3):
                    deng = nc.sync if (kq * 3 + pi) % 2 == 0 else nc.scalar
                    deng.dma_start(
                        w_all[:, pi, kq * 4:(kq + 1) * 4, :],
                        wds[pi][:, kq * 4:(kq + 1) * 4, :])
                if kq < 3:
                    deng = nc.scalar if kq % 2 == 0 else nc.sync
                    deng.dma_start(
                        xb0[:, 4 * (kq + 1):4 * (kq + 2), :],
                        d["xP"][:, 0, 4 * (kq + 1):4 * (kq + 2), :])
            convd_t = consts.tile([128, 3 * MPC * KCONV, 128], BF16)
            cosT_t = consts.tile([64, T], F32)
            ssin2_t = consts.tile([64, T], F32)
            normw_t = consts.tile([128, 2], F32)
            mask_t = consts.tile([128, 896], BF16)
            wo_t = worp.tile([128, MPC, D], BF16)

            ones_f = consts.tile([128, 1], F32)
            nc.vector.memset(ones_f, 1.0)
            ones_hd = consts.tile([128, 1], F32R)   # lhsT for partition sums
            nc.vector.tensor_copy(ones_hd, ones_f)
            ones_bf = consts.tile([128, 1], BF16)   # lhsT for bf16 e sums
            nc.vector.tensor_copy(ones_bf, ones_f)
            ones_1f = consts.tile([1, 128], F32)
            nc.vector.memset(ones_1f, 1.0)
            ones_1 = consts.tile([1, 128], F32R)    # lhsT for bcast over parts
            nc.vector.tensor_copy(ones_1, ones_1f)
            eps_t = consts.tile([1, 1], F32)
            nc.vector.memset(eps_t, EPS)
            ident_f = consts.tile([128, 128], F32)
            make_identity(nc, ident_f)
            ident_bf = consts.tile([128, 128], BF16)
            nc.vector.tensor_copy(ident_bf, ident_f)
            for r in (rawq, rawk, rawv):
                nc.vector.memset(r[:, :, 0:PAD], 0.0)

            # =============== Phase A: QKV projection for slice s ==========
            def phaseA(s):
                if s == 0:
                    xb = xb0
                else:
                    xb = xp.tile([128, KD, NT], BF16, name="xb")
                    deng = nc.sync if s % 2 == 0 else nc.scalar
                    deng.dma_start(xb, d["xP"][:, s])
                for grp in groups:
                    pst = [psa.tile([128, NT], F32, tag="acc",
                                    name=f"acc{gi}")
                           for gi in range(3)]
                    for k in range(KD):
                        for gi, (pi, m) in enumerate(grp):
                            nc.tensor.matmul(
                                pst[gi],
                                w_all[:, pi, k, m * 128:(m + 1) * 128],
                                xb[:, k, :],
                                start=(k == 0),
                                stop=(k == KD - 1),
                            )
                    for gi, (pi, m) in enumerate(grp):
                        dst = raws[pi][:, m,
                                       PAD + s * NT:PAD + (s + 1) * NT]
                        nc.scalar.copy(dst, pst[gi])

            # ====== Phase Ba: conv (PE diag matmuls) + SiLU for slice s ===
            def phaseBa(s):
                for m in range(MPC):
                    for pi in range(3):
                        cv = psb.tile([128, NT], F32, tag="sm", name="cv")
                        for j in range(KCONV):
                            nc.tensor.matmul(
                                cv,
                                convd_t[:, (pi * MPC + m) * KCONV + j, :],
                                raws[pi][:, m, s * NT + j:s * NT + j + NT],
                                start=(j == 0), stop=(j == KCONV - 1),
                            )
                        if pi < 2:
                            # silu back into raw, shifted 3 cols left
                            nc.scalar.activation(
                                raws[pi][:, m, s * NT:s * NT + NT], cv,
                                mybir.ActivationFunctionType.Silu)
                        else:
                            vv = vsilp.tile([128, NT], BF16, name="vv")
                            nc.scalar.activation(
                                vv, cv, mybir.ActivationFunctionType.Silu)
                            phaseBa.vv[(s, m)] = vv
            phaseBa.vv = {}

            # ====== vT(s): transpose v slice into [t, HD] layout ==========
            def phaseVT(s):
                for m in range(MPC):
                    vv = phaseBa.vv.pop((s, m))
                    ps_tr = psb.tile([128, NT], BF16, tag="sm", name="ps_tr")
                    for sub in range(NT // 128):
                        nc.tensor.transpose(
                            ps_tr[:, sub * 128:(sub + 1) * 128],
                            vv[:, sub * 128:(sub + 1) * 128], ident_bf)
                    nc.scalar.copy(
                        vtr[:, m, s * (NT // 128):(s + 1) * (NT // 128), :],
                        ps_tr.rearrange("p (s h) -> p s h", h=128))

            # ====== Phase Bb: RMS norm + RoPE; one item per (m, q|k) ======
            def phaseBb_item(s, m, pi):
                sl = slice(s * NT, (s + 1) * NT)
                fin = qfT if pi == 0 else kfT
                qs = raws[pi][:, m, s * NT:s * NT + NT]  # silu'd, bf16
                sq = scr.tile([128, NT], F32R, tag="sq", name="sq")
                nc.scalar.activation(
                    sq, qs, mybir.ActivationFunctionType.Square)
                ps_ss = psb.tile([1, NT], F32, tag="sm", name="ps_ss")
                nc.tensor.matmul(ps_ss, ones_hd, sq, start=True, stop=True)
                lms = scr.tile([1, NT], F32, tag="rst", name="lms")
                nc.scalar.activation(
                    lms, ps_ss, mybir.ActivationFunctionType.Ln,
                    scale=1.0 / HD, bias=eps_t)
                rr_r = scr.tile([1, NT], F32R, tag="rsr", name="rr_r")
                nc.scalar.activation(
                    rr_r, lms, mybir.ActivationFunctionType.Exp, scale=-0.5)
                ps_rb = psb.tile([128, NT], F32, tag="sm", name="ps_rb")
                nc.tensor.matmul(ps_rb, ones_1, rr_r, start=True, stop=True)
                qn = qnp.tile([128, NT], F32, tag="qn", name="qn")
                nc.vector.scalar_tensor_tensor(
                    qn, qs, normw_t[:, pi:pi + 1], ps_rb,
                    mybir.AluOpType.mult, mybir.AluOpType.mult,
                )
                # rope rows 0:RD (pass-through rows RD:128)
                rot2 = scr.tile([64, 2, NT], F32, tag="rot2")
                nc.gpsimd.tensor_mul(rot2[0:32, 1, :], qn[32:64],
                                     ssin2_t[32:64, sl])
                nc.gpsimd.tensor_mul(rot2[32:64, 1, :], qn[0:32],
                                     ssin2_t[0:32, sl])
                nc.vector.tensor_mul(rot2[:, 0, :], qn[0:RD],
                                     cosT_t[:, sl])
                nc.vector.tensor_add(fin[0:RD, m, sl], rot2[:, 0, :],
                                     rot2[:, 1, :])
                nc.gpsimd.tensor_copy(fin[RD:128, m, sl], qn[RD:128])

            def phaseBb_items(s):
                return [lambda m=m, pi=pi: phaseBb_item(s, m, pi)
                        for m in range(MPC) for pi in range(2)]

            # =============== Phase C: attention + output proj =============
            def phaseC_attn(tq, m):
                qsl = slice(tq * NT, (tq + 1) * NT)
                nch = 4 * tq + 4
                ps_attn = psa.tile([128, NT], F32, tag="acc", name="ps_attn")
                ps_sum = psone.tile([1, NT], F32, tag="one", name="ps_sum")

                def qk(tk):
                    ps_s = psb.tile([128, NT], F32, tag="sm", name="ps_s")
                    nc.tensor.matmul(
                        ps_s, kfT[:, m, tk * 128:(tk + 1) * 128],
                        qfT[:, m, qsl], start=True, stop=True)
                    e = expp.tile([128, NT], BF16, tag="e", name="e")
                    nc.scalar.activation(
                        e, ps_s, mybir.ActivationFunctionType.Exp,
                        scale=inv_sqrt_hd)
                    dd = tk * 128 - tq * NT
                    if dd >= 0:  # diagonal chunk: causal mask
                        nc.vector.tensor_mul(
                            e, e, mask_t[:, 384 - dd:896 - dd])
                    return e

                epipe = [qk(t) for t in range(min(3, nch))]
                for tk in range(nch):
                    if tk + 3 < nch:
                        epipe.append(qk(tk + 3))
                    e = epipe.pop(0)
                    nc.tensor.matmul(ps_attn, vtr[:, m, tk, :], e,
                                     start=(tk == 0), stop=(tk == nch - 1))
                    nc.tensor.matmul(ps_sum, ones_bf, e,
                                     start=(tk == 0), stop=(tk == nch - 1))
                # normalize: attn^T *= 1/sumexp (broadcast over parts)
                rr = scr.tile([1, NT], F32, tag="rst", name="rrs")
                nc.vector.reciprocal_approx_fast(rr, ps_sum)
                rr_r = scr.tile([1, NT], F32R, tag="rsr", name="rrs_r")
                nc.vector.tensor_copy(rr_r, rr)
                ps_rb = psb.tile([128, NT], F32, tag="sm", name="ps_rb")
                nc.tensor.matmul(ps_rb, ones_1, rr_r, start=True, stop=True)
                rb = scr.tile([128, NT], F32, tag="rbs")
                nc.scalar.copy(rb, ps_rb)
                am = attnp.tile([128, NT], BF16, tag="am", name="am")
                nc.vector.tensor_mul(am, ps_attn, rb)
                return am

            def phaseC_out(tq, attn_m, fillers):
                ost = None
                for i in range(D // 128):
                    if i % 4 == 0:
                        ost = ostp.tile([128, 4, NT], BF16, name="ost")
                    ps_o = psa.tile([128, NT], F32, tag="acc", name="ps_o")
                    for j in range(MPC):
                        nc.tensor.matmul(
                            ps_o, wo_t[:, j, i * 128:(i + 1) * 128],
                            attn_m[j],
                            start=(j == 0), stop=(j == MPC - 1))
                    nc.vector.tensor_copy(ost[:, i % 4, :], ps_o)
                    if i % 4 == 3:
                        nc.sync.dma_start(outP[:, tq, i // 4], ost)
                    if fillers:
                        fillers.pop(0)()
                while fillers:
                    fillers.pop(0)()

            # ================= emission schedule ==========================
            phaseA(0)
            nc.gpsimd.dma_start(convd_t, d["convd"])
            phaseA(1)
            phaseBa(0)
            nc.gpsimd.dma_start(cosT_t, d["trig"][0:64])
            nc.gpsimd.dma_start(ssin2_t, d["trig"][64:128])
            nc.gpsimd.dma_start(normw_t, d["normw"])
            phaseA(2)
            phaseBa(1)
            nc.gpsimd.dma_start(mask_t, d["maskb"])
            phaseA(3)
            phaseBa(2)
            nc.gpsimd.dma_start(wo_t, d["woP"])
            phaseBa(3)
            # Bb(0) interleaved with the v transposes (PE filler)
            bb0 = phaseBb_items(0)
            for s in range(NQ):
                bb0.pop(0)()
                phaseVT(s)
            for t in range(NQ):
                attn_m = [phaseC_attn(t, m) for m in range(MPC)]
                fillers = phaseBb_items(t + 1) if t + 1 < NQ else []
                phaseC_out(t, attn_m, fillers)

    nc.compile()
    return nc


def _prep_inputs(hidden_states, cos, sin, Wq, Wk, Wv, Wo,
                 conv_q_w, conv_k_w, conv_v_w, q_norm_w, k_norm_w):
    f = np.float32
    bf = ml_dtypes.bfloat16
    x = np.asarray(hidden_states, f)[0]            # [T, D]
    xT = x.T.astype(bf)                            # [D, T] bf16
    # pack: xP[p, s, k, t'] = xT[k*128+p, s*512+t']
    xP = np.ascontiguousarray(
        xT.reshape(KD, 128, NQ, NT).transpose(1, 2, 0, 3))

    def packw(W, sl):
        # wP[p, k, c] = W.T[k*128+p, sl][c]
        WT = np.asarray(W, f).T[:, sl].astype(bf)  # [D, CPC]
        return np.ascontiguousarray(
            WT.reshape(KD, 128, CPC).transpose(1, 0, 2))

    cosT = np.asarray(cos, f)[0].T                 # [RD, T]
    sinT = np.asarray(sin, f)[0].T
    trig = np.zeros((128, T), f)
    trig[0:RD] = cosT
    # ssin2 block (device rows 0:64): [0:32] = +sin[32:64], [32:64] = -sin[0:32]
    trig[RD:RD + 32] = sinT[32:64]
    trig[RD + 32:2 * RD] = -sinT[0:32]

    # causal mask strip: mask[kl, c] = 1.0 iff kl <= c - 384
    kl = np.arange(128, dtype=f)[:, None]
    cc = np.arange(896, dtype=f)[None, :]
    maskb = (kl <= cc - 384).astype(bf)

    nw = np.zeros((128, 2), f)
    nw[:, 0] = np.asarray(q_norm_w, f)
    nw[:, 1] = np.asarray(k_norm_w, f)

    WoTf = np.asarray(Wo, f).T.astype(bf)          # [D rows=c, D cols=dout]

    in_maps = []
    for c in range(NCORES):
        sl = slice(c * CPC, (c + 1) * CPC)
        # conv diagonal stationaries [128, (pi, m, j), 128]
        convd = np.zeros((128, 3 * MPC * KCONV, 128), bf)
        idx = np.arange(128)
        for pi, cw in enumerate((conv_q_w, conv_k_w, conv_v_w)):
            w = np.asarray(cw, f)[sl]              # [CPC, K]
            for m in range(MPC):
                for j in range(KCONV):
                    convd[idx, (pi * MPC + m) * KCONV + j, idx] = \
                        w[m * 128 + idx, j].astype(bf)
        woP = np.ascontiguousarray(
            WoTf[sl].reshape(MPC, 128, D).transpose(1, 0, 2))
        in_maps.append({
            "xP": xP,
            "wqP": packw(Wq, sl),
            "wkP": packw(Wk, sl),
            "wvP": packw(Wv, sl),
            "woP": woP,
            "trig": trig,
            "convd": np.ascontiguousarray(convd),
            "normw": nw,
            "maskb": maskb,
        })
    return in_maps


def kernel(hidden_states, cos, sin, Wq, Wk, Wv, Wo,
           conv_q_w, conv_k_w, conv_v_w, q_norm_w, k_norm_w,
           _trace=False):
    global _COMPILED
    if _COMPILED is None:
        _COMPILED = _build()
    nc = _COMPILED
    in_maps = _prep_inputs(hidden_states, cos, sin, Wq, Wk, Wv, Wo,
                           conv_q_w, conv_k_w, conv_v_w, q_norm_w, k_norm_w)
    res = bass_utils.run_bass_kernel_spmd(
        nc, in_maps, core_ids=list(range(NCORES)), trace=_trace)
    acc = np.zeros((D, T), np.float64)
    for r in res.results:
        arr = np.asarray(r["outP"], dtype=np.float32)  # [128,tq,blk4,i,t]
        # rows (blk4*4+i)*128+p, cols tq*512+t
        acc += arr.transpose(2, 3, 0, 1, 4).reshape(D, T)
    out = np.ascontiguousarray(acc.T.astype(np.float32))[None]
    if _trace:
        kernel._last_results = res
    return out


# revision 17
# speedup vs baseline: 1.1227x; 1.1227x over previous
"""Trainium2 Bass kernel for nn_Attention_34033320854122.

Dense transformer attention block: QKV proj -> causal depthwise conv+SiLU ->
per-head RMSNorm -> partial RoPE -> causal attention -> output projection.

Sharding: tensor-parallel over the 16 heads across 8 NeuronCores (2 heads =
256 channels per core). Each core computes q/k/v for its channels (full
contraction over D), runs attention for its 2 heads, and produces a partial
output projection (outT_partial = Wo[:, cols] @ attn_cols^T, bf16). The host
sums the 8 partials and transposes.

Numerics / fidelity notes (same conventions as the validated baseline):
- The reference negates the rotated RoPE sub-dim of BOTH q and k; the
  negation cancels in q . k and is skipped.
- softmax runs without max-subtraction (scores bounded well inside fp32 exp
  range for RMS-normed q/k at scale 1/sqrt(128)).
- Matmuls: QKV/Wo projections and attention PV in bf16; QK in float32r.
- The depthwise conv runs on the PE as 4 accumulating matmuls with
  diagonal stationary matrices diag(w[:, j]) built on the host.
- Activation functions are grouped by ACT table set (silu wave, then
  sqrt/exp interleaved per q-tile) to avoid table-load thrash.
- All DRAM tensors are packed host-side so each DMA is one contiguous run
  per partition (HWDGE issue time scales with descriptor count).
- The rms/rope chain for slice t+1 is emitted interleaved into the output
  projection of q-tile t so its cross-engine latency hides under PE work.
"""

import ml_dtypes
import numpy as np

import concourse.bacc as bacc
import concourse.tile as tile
import concourse.mybir as mybir
from concourse import bass_utils
from concourse.masks import make_identity

# Problem shape (hardcoded per contract)
B, T, D = 1, 2048, 2048
H, HD = 16, 128
RD = 64
KCONV = 4
EPS = 1e-5
NCORES = 8
CPC = D // NCORES      # channels per core = 256
MPC = CPC // HD        # head tiles per core = 2
NT = 512               # free-dim tile for matmuls
NQ = T // NT           # 4 q tiles
NKC = T // HD          # 16 key chunks of 128
KD = D // 128          # 16 contraction chunks
PAD = KCONV - 1        # causal conv history

F32 = mybir.dt.float32
F32R = mybir.dt.float32r
BF16 = mybir.dt.bfloat16

_COMPILED = None


def _build():
    nc = bacc.Bacc("TRN2", target_bir_lowering=False, debug=False,
                   num_devices=NCORES)

    d = {}
    # packed layouts: leading dim 128 = SBUF partition
    d["xP"] = nc.dram_tensor("xP", (128, NQ, KD, NT), BF16,
                             kind="ExternalInput").ap()
    d["wqP"] = nc.dram_tensor("wqP", (128, KD, CPC), BF16,
                              kind="ExternalInput").ap()
    d["wkP"] = nc.dram_tensor("wkP", (128, KD, CPC), BF16,
                              kind="ExternalInput").ap()
    d["wvP"] = nc.dram_tensor("wvP", (128, KD, CPC), BF16,
                              kind="ExternalInput").ap()
    d["woP"] = nc.dram_tensor("woP", (128, MPC, D), BF16,
                              kind="ExternalInput").ap()
    # trig: rows 0:64 = cos^T, rows 64:128 = sign-folded sin^T
    d["trig"] = nc.dram_tensor("trig", (128, T), F32, kind="ExternalInput").ap()
    # conv weights as diagonal stationaries [128, (pi,m,j)=24, 128]
    d["convd"] = nc.dram_tensor("convd", (128, 3 * MPC * KCONV, 128), BF16,
                                kind="ExternalInput").ap()
    # per-head norm weights [128, 2] (q, k)
    d["normw"] = nc.dram_tensor("normw", (128, 2), F32, kind="ExternalInput").ap()
    # causal mask strip [128, 896]: mask[kl, c] = 1.0 if kl <= c - 384
    d["maskb"] = nc.dram_tensor("maskb", (128, 896), BF16,
                                kind="ExternalInput").ap()
    # packed output: [p, tq, blk4, i, t'] -> out row (blk4*4+i)*128+p
    outP = nc.dram_tensor("outP", (128, NQ, 8, 2, NT), BF16,
                          kind="ExternalOutput").ap()

    inv_sqrt_hd = 1.0 / np.sqrt(HD)

    with tile.TileContext(nc) as tc:
        with (
            tc.tile_pool(name="consts", bufs=1) as consts,
            tc.tile_pool(name="raw", bufs=1) as rawp,
            tc.tile_pool(name="wqkv", bufs=1) as wqkvp,
            tc.tile_pool(name="wor", bufs=1) as worp,
            tc.tile_pool(name="final", bufs=1) as finalp,
            tc.tile_pool(name="xblk", bufs=2) as xp,
            tc.tile_pool(name="vsil", bufs=8) as vsilp,
            tc.tile_pool(name="scratch", bufs=2) as scr,
            tc.tile_pool(name="qn", bufs=2) as qnp,
            tc.tile_pool(name="sqp", bufs=1) as sqp,
            tc.tile_pool(name="rrp", bufs=1) as rrp,
            tc.tile_pool(name="exp", bufs=3) as expp,
            tc.tile_pool(name="attn", bufs=3) as attnp,
            tc.tile_pool(name="ostage", bufs=2) as ostp,
            tc.tile_pool(name="psa", bufs=3, space="PSUM") as psa,
            tc.tile_pool(name="psb", bufs=4, space="PSUM") as psb,
            tc.tile_pool(name="psone", bufs=1, space="PSUM") as psone,
        ):
            # ---- persistent buffers ----
            rawq = rawp.tile([128, MPC, T + PAD], BF16)
            rawk = rawp.tile([128, MPC, T + PAD], BF16)
            rawv = rawp.tile([128, MPC, T + PAD], BF16)
            qfT = finalp.tile([128, MPC, T], F32R)
            kfT = finalp.tile([128, MPC, T], F32R)
            vtr = finalp.tile([128, MPC, NKC, HD], BF16)
            w_all = wqkvp.tile([128, 3, KD, CPC], BF16)
            raws = (rawq, rawk, rawv)
            groups = [[(0, 0), (0, 1), (1, 0)], [(1, 1), (2, 0), (2, 1)]]

            # ---- startup DMAs: x tile 0 + weight k-blocks first ----
            xb0 = xp.tile([128, KD, NT], BF16, name="xb")
            nc.sync.dma_start(xb0[:, 0:4, :], d["xP"][:, 0, 0:4, :])
            wds = (d["wqP"], d["wkP"], d["wvP"])
            for kq in range(4):
                for pi in range(3):
                    deng = nc.sync if (kq * 3 + pi) % 2 == 0 else nc.scalar
                    deng.dma_start(
                        w_all[:, pi, kq * 4:(kq + 1) * 4, :],
                        wds[pi][:, kq * 4:(kq + 1) * 4, :])
                if kq < 3:
                    deng = nc.scalar if kq % 2 == 0 else nc.sync
                    deng.dma_start(
                        xb0[:, 4 * (kq + 1):4 * (kq + 2), :],
                        d["xP"][:, 0, 4 * (kq + 1):4 * (kq + 2), :])
            convd_t = consts.tile([128, 3 * MPC * KCONV, 128], BF16)
            cosT_t = consts.tile([64, T], F32)
            ssin2_t = consts.tile([64, T], F32)
            normw_t = consts.tile([128, 2], F32)
            mask_t = consts.tile([128, 896], BF16)
            wo_t = worp.tile([128, MPC, D], BF16)

            ones_f = consts.tile([128, 1], F32)
            nc.vector.memset(ones_f, 1.0)
            ones_hd = consts.tile([128, 1], F32R)   # lhsT for partition sums
            nc.vector.tensor_copy(ones_hd, ones_f)
            ones_bf = consts.tile([128, 1], BF16)   # lhsT for bf16 e sums
            nc.vector.tensor_copy(ones_bf, ones_f)
            ones_1f = consts.tile([1, 128], F32)
            nc.vector.memset(ones_1f, 1.0)
            ones_1 = consts.tile([1, 128], F32R)    # lhsT for bcast over parts
            nc.vector.tensor_copy(ones_1, ones_1f)
            eps_t = consts.tile([1, 1], F32)
            nc.vector.memset(eps_t, EPS)
            ident_f = consts.tile([128, 128], F32)
            make_identity(nc, ident_f)
            ident_bf = consts.tile([128, 128], BF16)
            nc.vector.tensor_copy(ident_bf, ident_f)
            for r in (rawq, rawk, rawv):
                nc.vector.memset(r[:, :, 0:PAD], 0.0)

            # =============== Phase A: QKV projection for slice s ==========
            def phaseA(s):
                if s == 0:
                    xb = xb0
                else:
                    xb = xp.tile([128, KD, NT], BF16, name="xb")
                    deng = nc.sync if s % 2 == 0 else nc.scalar
                    deng.dma_start(xb, d["xP"][:, s])
                for grp in groups:
                    pst = [psa.tile([128, NT], F32, tag="acc",
                                    name=f"acc{gi}")
                           for gi in range(3)]
                    for k in range(KD):
                        for gi, (pi, m) in enumerate(grp):
                            nc.tensor.matmul(
                                pst[gi],
                                w_all[:, pi, k, m * 128:(m + 1) * 128],
                                xb[:, k, :],
                                start=(k == 0),
                                stop=(k == KD - 1),
                            )
                    for gi, (pi, m) in enumerate(grp):
                        dst = raws[pi][:, m,
                                       PAD + s * NT:PAD + (s + 1) * NT]
                        nc.scalar.copy(dst, pst[gi])

            # ====== Phase Ba: conv (PE diag matmuls) + SiLU for slice s ===
            def phaseBa(s):
                for m in range(MPC):
                    for pi in range(3):
                        cv = psb.tile([128, NT], F32, tag="sm", name="cv")
                        for j in range(KCONV):
                            nc.tensor.matmul(
                                cv,
                                convd_t[:, (pi * MPC + m) * KCONV + j, :],
                                raws[pi][:, m, s * NT + j:s * NT + j + NT],
                                start=(j == 0), stop=(j == KCONV - 1),
                            )
                        if pi < 2:
                            # silu back into raw, shifted 3 cols left
                            nc.scalar.activation(
                                raws[pi][:, m, s * NT:s * NT + NT], cv,
                                mybir.ActivationFunctionType.Silu)
                        else:
                            vv = vsilp.tile([128, NT], BF16, name="vv")
                            nc.scalar.activation(
                                vv, cv, mybir.ActivationFunctionType.Silu)
                            phaseBa.vv[(s, m)] = vv
            phaseBa.vv = {}

            # ====== vT(s): transpose v slice into [t, HD] layout ==========
            def phaseVT(s):
                for m in range(MPC):
                    vv = phaseBa.vv.pop((s, m))
                    ps_tr = psb.tile([128, NT], BF16, tag="sm", name="ps_tr")
                    for sub in range(NT // 128):
                        nc.tensor.transpose(
                            ps_tr[:, sub * 128:(sub + 1) * 128],
                            vv[:, sub * 128:(sub + 1) * 128], ident_bf)
                    nc.scalar.copy(
                        vtr[:, m, s * (NT // 128):(s + 1) * (NT // 128), :],
                        ps_tr.rearrange("p (s h) -> p s h", h=128))

            # ====== Phase Bb: RMS norm + RoPE, staged for pipelining ======
            # stage1: squares (scalar only).  stage23: partition-sum MM +
            # sqrt + recip + f32r round (per item).  stage45: bcast MM +
            # qn + rope (per item).
            bbst = {}

            def bb_stage1(s):
                sq4 = sqp.tile([128, 4, NT], BF16, name="sq4")
                for m in range(MPC):
                    for pi in range(2):
                        qs = raws[pi][:, m, s * NT:s * NT + NT]
                        nc.scalar.activation(
                            sq4[:, m * 2 + pi, :], qs,
                            mybir.ActivationFunctionType.Square)
                bbst[s] = [sq4]

            def bb_stage23(s):
                sq4 = bbst[s][0]
                rr4 = rrp.tile([1, 4, NT], F32R, name="rr4")
                for m in range(MPC):
                    for pi in range(2):
                        ps_ss = psb.tile([1, NT], F32, tag="sm",
                                         name="ps_ss")
                        nc.tensor.matmul(ps_ss, ones_bf,
                                         sq4[:, m * 2 + pi, :],
                                         start=True, stop=True)
                        rstd = scr.tile([1, NT], F32, tag="rst", name="rstd")
                        nc.scalar.activation(
                            rstd, ps_ss, mybir.ActivationFunctionType.Sqrt,
                            scale=1.0 / HD, bias=eps_t)
                        rr = scr.tile([1, NT], F32, tag="rst", name="rr")
                        nc.vector.reciprocal_approx_fast(rr, rstd)
                        nc.vector.tensor_copy(rr4[:, m * 2 + pi, :], rr)
                bbst[s].append(rr4)

            def bb_stage45(s):
                sl = slice(s * NT, (s + 1) * NT)
                rr4 = bbst.pop(s)[1]
                for m in range(MPC):
                    for pi in range(2):
                        fin = qfT if pi == 0 else kfT
                        qs = raws[pi][:, m, s * NT:s * NT + NT]
                        ps_rb = psb.tile([128, NT], F32, tag="sm",
                                         name="ps_rb")
                        nc.tensor.matmul(ps_rb, ones_1,
                                         rr4[:, m * 2 + pi, :],
                                         start=True, stop=True)
                        qn = qnp.tile([128, NT], F32, tag="qn", name="qn")
                        nc.vector.scalar_tensor_tensor(
                            qn, qs, normw_t[:, pi:pi + 1], ps_rb,
                            mybir.AluOpType.mult, mybir.AluOpType.mult,
                        )
                        # rope rows 0:RD (pass-through rows RD:128)
                        rot2 = scr.tile([64, 2, NT], F32, tag="rot2")
                        nc.gpsimd.tensor_mul(rot2[0:32, 1, :], qn[32:64],
                                             ssin2_t[32:64, sl])
                        nc.vector.tensor_mul(rot2[32:64, 1, :], qn[0:32],
                                             ssin2_t[0:32, sl])
                        nc.vector.tensor_mul(rot2[:, 0, :], qn[0:RD],
                                             cosT_t[:, sl])
                        nc.vector.tensor_add(fin[0:RD, m, sl],
                                             rot2[:, 0, :], rot2[:, 1, :])
                        nc.scalar.copy(fin[RD:128, m, sl], qn[RD:128])


            # =============== Phase C: attention + output proj =============
            def phaseC_attn(tq, m):
                qsl = slice(tq * NT, (tq + 1) * NT)
                nch = 4 * tq + 4
                ps_attn = psa.tile([128, NT], F32, tag="acc", name="ps_attn")
                ps_sum = psone.tile([1, NT], F32, tag="one", name="ps_sum")

                def qk(tk):
                    ps_s = psb.tile([128, NT], F32, tag="sm", name="ps_s")
                    nc.tensor.matmul(
                        ps_s, kfT[:, m, tk * 128:(tk + 1) * 128],
                        qfT[:, m, qsl], start=True, stop=True)
                    e = expp.tile([128, NT], BF16, tag="e", name="e")
                    nc.scalar.activation(
                        e, ps_s, mybir.ActivationFunctionType.Exp,
                        scale=inv_sqrt_hd)
                    dd = tk * 128 - tq * NT
                    if dd >= 0:  # diagonal chunk: causal mask
                        nc.vector.tensor_mul(
                            e, e, mask_t[:, 384 - dd:896 - dd])
                    return e

                epipe = [qk(t) for t in range(min(3, nch))]
                for tk in range(nch):
                    if tk + 3 < nch:
                        epipe.append(qk(tk + 3))
                    e = epipe.pop(0)
                    nc.tensor.matmul(ps_attn, vtr[:, m, tk, :], e,
                                     start=(tk == 0), stop=(tk == nch - 1))
                    nc.tensor.matmul(ps_sum, ones_bf, e,
                                     start=(tk == 0), stop=(tk == nch - 1))
                # normalize: attn^T *= 1/sumexp (broadcast over parts)
                rr = scr.tile([1, NT], F32, tag="rst", name="rrs")
                nc.vector.reciprocal_approx_fast(rr, ps_sum)
                rr_r = scr.tile([1, NT], F32R, tag="rsr", name="rrs_r")
                nc.vector.tensor_copy(rr_r, rr)
                ps_rb = psb.tile([128, NT], F32, tag="sm", name="ps_rb")
                nc.tensor.matmul(ps_rb, ones_1, rr_r, start=True, stop=True)
                rb = scr.tile([128, NT], F32, tag="rbs")
                nc.scalar.copy(rb, ps_rb)
                am = attnp.tile([128, NT], BF16, tag="am", name="am")
                nc.vector.tensor_mul(am, ps_attn, rb)
                return am

            def phaseC_out(tq, attn_m):
                ost = None
                for i in range(D // 128):
                    if i % 2 == 0:
                        ost = ostp.tile([128, 2, NT], BF16, name="ost")
                    ps_o = psa.tile([128, NT], F32, tag="acc", name="ps_o")
                    for j in range(MPC):
                        nc.tensor.matmul(
                            ps_o, wo_t[:, j, i * 128:(i + 1) * 128],
                            attn_m[j],
                            start=(j == 0), stop=(j == MPC - 1))
                    if i % 2 == 0:
                        nc.vector.tensor_copy(ost[:, 0, :], ps_o)
                    else:
                        nc.scalar.copy(ost[:, 1, :], ps_o)
                        nc.sync.dma_start(outP[:, tq, i // 2], ost)

            # ================= emission schedule ==========================
            phaseA(0)
            nc.gpsimd.dma_start(convd_t, d["convd"])
            phaseA(1)
            phaseBa(0)
            nc.gpsimd.dma_start(cosT_t, d["trig"][0:64])
            nc.gpsimd.dma_start(ssin2_t, d["trig"][64:128])
            nc.gpsimd.dma_start(normw_t, d["normw"])
            phaseA(2)
            phaseBa(1)
            nc.gpsimd.dma_start(mask_t, d["maskb"])
            phaseA(3)
            phaseBa(2)
            nc.gpsimd.dma_start(wo_t, d["woP"])
            phaseBa(3)
            # Bb(0) staged and interleaved with the v transposes
            bb_stage1(0)
            phaseVT(0)
            phaseVT(1)
            bb_stage23(0)
            phaseVT(2)
            phaseVT(3)
            bb_stage45(0)
            for t in range(NQ):
                nxt = t + 1 < NQ
                if nxt:
                    bb_stage1(t + 1)
                am0 = phaseC_attn(t, 0)
                if nxt:
                    bb_stage23(t + 1)
                am1 = phaseC_attn(t, 1)
                if nxt:
                    bb_stage45(t + 1)
                phaseC_out(t, [am0, am1])

    nc.compile()
    return nc


def _prep_inputs(hidden_states, cos, sin, Wq, Wk, Wv, Wo,
                 conv_q_w, conv_k_w, conv_v_w, q_norm_w, k_norm_w):
    f = np.float32
    bf = ml_dtypes.bfloat16
    x = np.asarray(hidden_states, f)[0]            # [T, D]
    xT = x.T.astype(bf)                            # [D, T] bf16
    # pack: xP[p, s, k, t'] = xT[k*128+p, s*512+t']
    xP = np.ascontiguousarray(
        xT.reshape(KD, 128, NQ, NT).transpose(1, 2, 0, 3))

    def packw(W, sl):
        # wP[p, k, c] = W.T[k*128+p, sl][c]
        WT = np.asarray(W, f).T[:, sl].astype(bf)  # [D, CPC]
        return np.ascontiguousarray(
            WT.reshape(KD, 128, CPC).transpose(1, 0, 2))

    cosT = np.asarray(cos, f)[0].T                 # [RD, T]
    sinT = np.asarray(sin, f)[0].T
    trig = np.zeros((128, T), f)
    trig[0:RD] = cosT
    # ssin2 block (device rows 0:64): [0:32] = +sin[32:64], [32:64] = -sin[0:32]
    trig[RD:RD + 32] = sinT[32:64]
    trig[RD + 32:2 * RD] = -sinT[0:32]

    # causal mask strip: mask[kl, c] = 1.0 iff kl <= c - 384
    kl = np.arange(128, dtype=f)[:, None]
    cc = np.arange(896, dtype=f)[None, :]
    maskb = (kl <= cc - 384).astype(bf)

    nw = np.zeros((128, 2), f)
    nw[:, 0] = np.asarray(q_norm_w, f)
    nw[:, 1] = np.asarray(k_norm_w, f)

    WoTf = np.asarray(Wo, f).T.astype(bf)          # [D rows=c, D cols=dout]

    in_maps = []
    for c in range(NCORES):
        sl = slice(c * CPC, (c + 1) * CPC)
        # conv diagonal stationaries [128, (pi, m, j), 128]
        convd = np.zeros((128, 3 * MPC * KCONV, 128), bf)
        idx = np.arange(128)
        for pi, cw in enumerate((conv_q_w, conv_k_w, conv_v_w)):
            w = np.asarray(cw, f)[sl]              # [CPC, K]
            for m in range(MPC):
                for j in range(KCONV):
                    convd[idx, (pi * MPC + m) * KCONV + j, idx] = \
                        w[m * 128 + idx, j].astype(bf)
        woP = np.ascontiguousarray(
            WoTf[sl].reshape(MPC, 128, D).transpose(1, 0, 2))
        in_maps.append({
            "xP": xP,
            "wqP": packw(Wq, sl),
            "wkP": packw(Wk, sl),
            "wvP": packw(Wv, sl),
            "woP": woP,
            "trig": trig,
            "convd": np.ascontiguousarray(convd),
            "normw": nw,
            "maskb": maskb,
        })
    return in_maps


def kernel(hidden_states, cos, sin, Wq, Wk, Wv, Wo,
           conv_q_w, conv_k_w, conv_v_w, q_norm_w, k_norm_w,
           _trace=False):
    global _COMPILED
    if _COMPILED is None:
        _COMPILED = _build()
    nc = _COMPILED
    in_maps = _prep_inputs(hidden_states, cos, sin, Wq, Wk, Wv, Wo,
                           conv_q_w, conv_k_w, conv_v_w, q_norm_w, k_norm_w)
    res = bass_utils.run_bass_kernel_spmd(
        nc, in_maps, core_ids=list(range(NCORES)), trace=_trace)
    acc = np.zeros((D, T), np.float64)
    for r in res.results:
        arr = np.asarray(r["outP"], dtype=np.float32)  # [128,tq,blk4,i,t]
        # rows (blk4*4+i)*128+p, cols tq*512+t
        acc += arr.transpose(2, 3, 0, 1, 4).reshape(D, T)
    out = np.ascontiguousarray(acc.T.astype(np.float32))[None]
    if _trace:
        kernel._last_results = res
    return out


# revision 20
# speedup vs baseline: 1.1474x; 1.0220x over previous
"""Trainium2 Bass kernel for nn_Attention_34033320854122.

Dense transformer attention block: QKV proj -> causal depthwise conv+SiLU ->
per-head RMSNorm -> partial RoPE -> causal attention -> output projection.

Sharding: tensor-parallel over the 16 heads across 8 NeuronCores (2 heads =
256 channels per core). Each core computes q/k/v for its channels (full
contraction over D), runs attention for its 2 heads, and produces a partial
output projection (outT_partial = Wo[:, cols] @ attn_cols^T, bf16). The host
sums the 8 partials and transposes.

Numerics / fidelity notes (same conventions as the validated baseline):
- The reference negates the rotated RoPE sub-dim of BOTH q and k; the
  negation cancels in q . k and is skipped.
- softmax runs without max-subtraction (scores bounded well inside fp32 exp
  range for RMS-normed q/k at scale 1/sqrt(128)).
- Matmuls: QKV/Wo projections and attention PV in bf16; QK in float32r.
- The depthwise conv runs on the PE as 4 accumulating matmuls with
  diagonal stationary matrices diag(w[:, j]) built on the host.
- RMSNorm rescale commutes past RoPE (per-position scalar), so the trig
  products are computed on the SiLU output early (hiding on the otherwise
  idle Vector engine during the projection phase) and the rstd * norm_w
  scale is applied at the end via two scalar_tensor_tensor ops.
- Activation table sets: silu wave first, then only {square, sqrt} around
  {exp}, with sqrt batches scheduled at q-tile starts (before exps are
  ready) to avoid per-op table-load thrash.
- All DRAM tensors are packed host-side so each DMA is one contiguous run
  per partition (HWDGE issue time scales with descriptor count).
"""

import ml_dtypes
import numpy as np

import concourse.bacc as bacc
import concourse.tile as tile
import concourse.mybir as mybir
from concourse import bass_utils
from concourse.masks import make_identity

# Problem shape (hardcoded per contract)
B, T, D = 1, 2048, 2048
H, HD = 16, 128
RD = 64
KCONV = 4
EPS = 1e-5
NCORES = 8
CPC = D // NCORES      # channels per core = 256
MPC = CPC // HD        # head tiles per core = 2
NT = 512               # free-dim tile for matmuls
NQ = T // NT           # 4 q tiles
NKC = T // HD          # 16 key chunks of 128
KD = D // 128          # 16 contraction chunks
PAD = KCONV - 1        # causal conv history

F32 = mybir.dt.float32
F32R = mybir.dt.float32r
BF16 = mybir.dt.bfloat16

_COMPILED = None


def _build():
    nc = bacc.Bacc("TRN2", target_bir_lowering=False, debug=False,
                   num_devices=NCORES)

    d = {}
    # packed layouts: leading dim 128 = SBUF partition
    d["xP"] = nc.dram_tensor("xP", (128, NQ, KD, NT), BF16,
                             kind="ExternalInput").ap()
    d["wqP"] = nc.dram_tensor("wqP", (128, KD, CPC), BF16,
                              kind="ExternalInput").ap()
    d["wkP"] = nc.dram_tensor("wkP", (128, KD, CPC), BF16,
                              kind="ExternalInput").ap()
    d["wvP"] = nc.dram_tensor("wvP", (128, KD, CPC), BF16,
                              kind="ExternalInput").ap()
    d["woP"] = nc.dram_tensor("woP", (128, MPC, D), BF16,
                              kind="ExternalInput").ap()
    # trig: rows 0:64 = cos^T, rows 64:128 = sign-folded sin^T
    d["trig"] = nc.dram_tensor("trig", (128, T), F32, kind="ExternalInput").ap()
    # conv weights as diagonal stationaries [128, (pi,m,j)=24, 128]
    d["convd"] = nc.dram_tensor("convd", (128, 3 * MPC * KCONV, 128), BF16,
                                kind="ExternalInput").ap()
    # per-head norm weights [128, 2] (q, k)
    d["normw"] = nc.dram_tensor("normw", (128, 2), F32, kind="ExternalInput").ap()
    # causal mask strip [128, 896]: mask[kl, c] = 1.0 if kl <= c - 384
    d["maskb"] = nc.dram_tensor("maskb", (128, 896), BF16,
                                kind="ExternalInput").ap()
    # packed output: [p, tq, blk2, i, t'] -> out row (blk2*2+i)*128+p
    outP = nc.dram_tensor("outP", (128, NQ, 8, 2, NT), BF16,
                          kind="ExternalOutput").ap()

    inv_sqrt_hd = 1.0 / np.sqrt(HD)

    with tile.TileContext(nc) as tc:
        with (
            tc.tile_pool(name="consts", bufs=1) as consts,
            tc.tile_pool(name="raw", bufs=1) as rawp,
            tc.tile_pool(name="wor", bufs=1) as worp,
            tc.tile_pool(name="final", bufs=1) as finalp,
            tc.tile_pool(name="vsil", bufs=8) as vsilp,
            tc.tile_pool(name="rc", bufs=3) as rcp,
            tc.tile_pool(name="scratch", bufs=2) as scr,
            tc.tile_pool(name="psa", bufs=3, space="PSUM") as psa,
            tc.tile_pool(name="psb", bufs=4, space="PSUM") as psb,
            tc.tile_pool(name="psone", bufs=1, space="PSUM") as psone,
        ):
            # ---- persistent buffers ----
            rawq = rawp.tile([128, MPC, T + PAD], BF16)
            rawk = rawp.tile([128, MPC, T + PAD], BF16)
            rawv = rawp.tile([128, MPC, T + PAD], BF16)
            qfT = finalp.tile([128, MPC, T], F32R)
            kfT = finalp.tile([128, MPC, T], F32R)
            vtr = finalp.tile([128, MPC, NKC, HD], BF16)
            raws = (rawq, rawk, rawv)
            groups = [[(0, 0), (0, 1), (1, 0)], [(1, 1), (2, 0), (2, 1)]]

            convd_t = consts.tile([128, 3 * MPC * KCONV, 128], BF16)
            cosT_t = consts.tile([64, T], F32)
            ssin2_t = consts.tile([64, T], F32)
            normw_t = consts.tile([128, 2], F32)
            mask_t = consts.tile([128, 896], BF16)
            wo_t = worp.tile([128, MPC, D], BF16)

            ones_f = consts.tile([128, 1], F32)
            nc.vector.memset(ones_f, 1.0)
            ones_bf = consts.tile([128, 1], BF16)   # lhsT for bf16 col sums
            nc.vector.tensor_copy(ones_bf, ones_f)
            ones_1f = consts.tile([1, 128], F32)
            nc.vector.memset(ones_1f, 1.0)
            ones_1 = consts.tile([1, 128], F32R)    # lhsT for bcast over parts
            nc.vector.tensor_copy(ones_1, ones_1f)
            eps_t = consts.tile([1, 1], F32)
            nc.vector.memset(eps_t, EPS)
            ident_f = consts.tile([128, 128], F32)
            make_identity(nc, ident_f)
            ident_bf = consts.tile([128, 128], BF16)
            nc.vector.tensor_copy(ident_bf, ident_f)
            for r in (rawq, rawk, rawv):
                nc.vector.memset(r[:, :, 0:PAD], 0.0)

            # ====== rope trig part on the silu output (rstd-independent):
            # rc[(s,m,pi)] rows 0:RD = qs*cos + rotate_half(qs)*ssin2
            def stage_trig(s):
                sl = slice(s * NT, (s + 1) * NT)
                for m in range(MPC):
                    for pi in range(2):
                        qs = raws[pi][:, m, s * NT:s * NT + NT]
                        rot2 = scr.tile([64, 2, NT], F32, tag="rot2")
                        nc.gpsimd.tensor_mul(rot2[0:32, 1, :], qs[32:64],
                                             ssin2_t[32:64, sl])
                        nc.gpsimd.tensor_mul(rot2[32:64, 1, :], qs[0:32],
                                             ssin2_t[0:32, sl])
                        nc.vector.tensor_mul(rot2[:, 0, :], qs[0:RD],
                                             cosT_t[:, sl])
                        rc = rcp.tile([64, NT], F32, tag=f"rc{m}{pi}",
                                      name="rc")
                        nc.vector.tensor_add(rc, rot2[:, 0, :],
                                             rot2[:, 1, :])
                        stage_trig.rc[(s, m, pi)] = rc
            stage_trig.rc = {}

            # =============== Phase A: QKV projection for slice s ==========
            def phaseA(s, w_all, xp, xb0=None):
                if s == 0:
                    xb = xb0
                else:
                    xb = xp.tile([128, KD, NT], BF16, name="xb")
                    deng = nc.sync if s % 2 == 0 else nc.scalar
                    deng.dma_start(xb, d["xP"][:, s])
                for grp in groups:
                    pst = [psa.tile([128, NT], F32, tag="acc",
                                    name=f"acc{gi}")
                           for gi in range(3)]
                    for k in range(KD):
                        for gi, (pi, m) in enumerate(grp):
                            nc.tensor.matmul(
                                pst[gi],
                                w_all[:, pi, k, m * 128:(m + 1) * 128],
                                xb[:, k, :],
                                start=(k == 0),
                                stop=(k == KD - 1),
                            )
                    for gi, (pi, m) in enumerate(grp):
                        dst = raws[pi][:, m,
                                       PAD + s * NT:PAD + (s + 1) * NT]
                        nc.scalar.copy(dst, pst[gi])

            # ====== Phase Ba: conv (PE diag matmuls) + SiLU for slice s ===
            def phaseBa(s):
                for m in range(MPC):
                    for pi in range(3):
                        cv = psb.tile([128, NT], F32, tag="sm", name="cv")
                        for j in range(KCONV):
                            nc.tensor.matmul(
                                cv,
                                convd_t[:, (pi * MPC + m) * KCONV + j, :],
                                raws[pi][:, m, s * NT + j:s * NT + j + NT],
                                start=(j == 0), stop=(j == KCONV - 1),
                            )
                        if pi < 2:
                            # silu back into raw, shifted 3 cols left
                            nc.scalar.activation(
                                raws[pi][:, m, s * NT:s * NT + NT], cv,
                                mybir.ActivationFunctionType.Silu)
                        else:
                            vv = vsilp.tile([128, NT], BF16, name="vv")
                            nc.scalar.activation(
                                vv, cv, mybir.ActivationFunctionType.Silu)
                            phaseBa.vv[(s, m)] = vv
            phaseBa.vv = {}

            # ====== vT(s): transpose v slice into [t, HD] layout ==========
            def phaseVT(s):
                for m in range(MPC):
                    vv = phaseBa.vv.pop((s, m))
                    ps_tr = psb.tile([128, NT], BF16, tag="sm", name="ps_tr")
                    for sub in range(NT // 128):
                        nc.tensor.transpose(
                            ps_tr[:, sub * 128:(sub + 1) * 128],
                            vv[:, sub * 128:(sub + 1) * 128], ident_bf)
                    nc.scalar.copy(
                        vtr[:, m, s * (NT // 128):(s + 1) * (NT // 128), :],
                        ps_tr.rearrange("p (s h) -> p s h", h=128))

            # ========== emission: projection + conv + silu + trig =========
            with (
                tc.tile_pool(name="wqkv", bufs=1) as wqkvp,
                tc.tile_pool(name="xblk", bufs=2) as xp,
            ):
                w_all = wqkvp.tile([128, 3, KD, CPC], BF16)
                xb0 = xp.tile([128, KD, NT], BF16, name="xb")
                nc.sync.dma_start(xb0[:, 0:4, :], d["xP"][:, 0, 0:4, :])
                wds = (d["wqP"], d["wkP"], d["wvP"])
                for kq in range(4):
                    for pi in range(3):
                        deng = nc.sync if (kq * 3 + pi) % 2 == 0 \
                            else nc.scalar
                        deng.dma_start(
                            w_all[:, pi, kq * 4:(kq + 1) * 4, :],
                            wds[pi][:, kq * 4:(kq + 1) * 4, :])
                    if kq < 3:
                        deng = nc.scalar if kq % 2 == 0 else nc.sync
                        deng.dma_start(
                            xb0[:, 4 * (kq + 1):4 * (kq + 2), :],
                            d["xP"][:, 0, 4 * (kq + 1):4 * (kq + 2), :])

                phaseA(0, w_all, xp, xb0)
                nc.gpsimd.dma_start(convd_t, d["convd"])
                phaseA(1, w_all, xp)
                phaseBa(0)
                nc.gpsimd.dma_start(cosT_t, d["trig"][0:64])
                nc.gpsimd.dma_start(ssin2_t, d["trig"][64:128])
                nc.gpsimd.dma_start(normw_t, d["normw"])
                phaseA(2, w_all, xp)
                stage_trig(0)
                phaseBa(1)
                nc.gpsimd.dma_start(mask_t, d["maskb"])
                phaseA(3, w_all, xp)
                stage_trig(1)
                phaseBa(2)
                nc.gpsimd.dma_start(wo_t, d["woP"])
                phaseBa(3)
                stage_trig(2)

            # ====== Phase Bb: RMS norm, staged for pipelining =============
            # stage1: squares (scalar).  stage23: partition-sum MM + sqrt +
            # recip + f32r round.  stage45: bcast MM + 2 STTs into qfT/kfT.
            with (
                tc.tile_pool(name="sqp", bufs=1) as sqp,
                tc.tile_pool(name="rsd", bufs=1) as rsdp,
                tc.tile_pool(name="exp", bufs=3) as expp,
                tc.tile_pool(name="attn", bufs=3) as attnp,
                tc.tile_pool(name="ostage", bufs=2) as ostp,
            ):
                bbst = {}

                def bb_stage1(s):
                    sq4 = sqp.tile([128, 4, NT], BF16, name="sq4")
                    for m in range(MPC):
                        for pi in range(2):
                            qs = raws[pi][:, m, s * NT:s * NT + NT]
                            nc.scalar.activation(
                                sq4[:, m * 2 + pi, :], qs,
                                mybir.ActivationFunctionType.Square)
                    bbst[s] = [sq4]

                def bb_stage23(s):
                    sq4 = bbst[s][0]
                    rsd4 = rsdp.tile([1, 4, NT], F32, name="rsd4")
                    rr4f = rsdp.tile([1, 4, NT], F32, name="rr4f")
                    rr4 = rsdp.tile([1, 4, NT], F32R, name="rr4")
                    for k in range(4):
                        ps_ss = psb.tile([1, NT], F32, tag="sm",
                                         name="ps_ss")
                        nc.tensor.matmul(ps_ss, ones_bf, sq4[:, k, :],
                                         start=True, stop=True)
                        nc.scalar.activation(
                            rsd4[:, k, :], ps_ss,
                            mybir.ActivationFunctionType.Sqrt,
                            scale=1.0 / HD, bias=eps_t)
                        nc.vector.reciprocal_approx_fast(
                            rr4f[:, k, :], rsd4[:, k, :])
                        nc.vector.tensor_copy(rr4[:, k, :], rr4f[:, k, :])
                    bbst[s].append(rr4)

                def bb_stage45(s):
                    sl = slice(s * NT, (s + 1) * NT)
                    rr4 = bbst.pop(s)[1]
                    for m in range(MPC):
                        for pi in range(2):
                            fin = qfT if pi == 0 else kfT
                            qs = raws[pi][:, m, s * NT:s * NT + NT]
                            rc = stage_trig.rc.pop((s, m, pi))
                            ps_rb = psb.tile([128, NT], F32, tag="sm",
                                             name="ps_rb")
                            nc.tensor.matmul(ps_rb, ones_1,
                                             rr4[:, m * 2 + pi, :],
                                             start=True, stop=True)
                            nc.vector.scalar_tensor_tensor(
                                fin[0:RD, m, sl], rc,
                                normw_t[0:RD, pi:pi + 1], ps_rb[0:RD, :],
                                mybir.AluOpType.mult, mybir.AluOpType.mult,
                            )
                            nc.vector.scalar_tensor_tensor(
                                fin[RD:128, m, sl], qs[RD:128],
                                normw_t[RD:128, pi:pi + 1],
                                ps_rb[RD:128, :],
                                mybir.AluOpType.mult, mybir.AluOpType.mult,
                            )

                # =========== Phase C: attention + output proj =============
                def phaseC_attn(tq, m):
                    qsl = slice(tq * NT, (tq + 1) * NT)
                    nch = 4 * tq + 4
                    ps_attn = psa.tile([128, NT], F32, tag="acc",
                                       name="ps_attn")
                    ps_sum = psone.tile([1, NT], F32, tag="one",
                                        name="ps_sum")

                    def qk(tk):
                        ps_s = psb.tile([128, NT], F32, tag="sm",
                                        name="ps_s")
                        nc.tensor.matmul(
                            ps_s, kfT[:, m, tk * 128:(tk + 1) * 128],
                            qfT[:, m, qsl], start=True, stop=True)
                        e = expp.tile([128, NT], BF16, tag="e", name="e")
                        nc.scalar.activation(
                            e, ps_s, mybir.ActivationFunctionType.Exp,
                            scale=inv_sqrt_hd)
                        dd = tk * 128 - tq * NT
                        if dd >= 0:  # diagonal chunk: causal mask
                            nc.vector.tensor_mul(
                                e, e, mask_t[:, 384 - dd:896 - dd])
                        return e

                    epipe = [qk(t) for t in range(min(3, nch))]
                    for tk in range(nch):
                        if tk + 3 < nch:
                            epipe.append(qk(tk + 3))
                        e = epipe.pop(0)
                        nc.tensor.matmul(ps_attn, vtr[:, m, tk, :], e,
                                         start=(tk == 0),
                                         stop=(tk == nch - 1))
                        nc.tensor.matmul(ps_sum, ones_bf, e,
                                         start=(tk == 0),
                                         stop=(tk == nch - 1))
                    # normalize: attn^T *= 1/sumexp (broadcast over parts)
                    rr = scr.tile([1, NT], F32, tag="rst", name="rrs")
                    nc.vector.reciprocal_approx_fast(rr, ps_sum)
                    rr_r = scr.tile([1, NT], F32R, tag="rsr", name="rrs_r")
                    nc.vector.tensor_copy(rr_r, rr)
                    ps_rb = psb.tile([128, NT], F32, tag="sm", name="ps_rb")
                    nc.tensor.matmul(ps_rb, ones_1, rr_r, start=True,
                                     stop=True)
                    rb = scr.tile([128, NT], F32, tag="rbs")
                    nc.scalar.copy(rb, ps_rb)
                    am = attnp.tile([128, NT], BF16, tag="am", name="am")
                    nc.vector.tensor_mul(am, ps_attn, rb)
                    return am

                def phaseC_out(tq, attn_m):
                    ost = None
                    for i in range(D // 128):
                        if i % 2 == 0:
                            ost = ostp.tile([128, 2, NT], BF16, name="ost")
                        ps_o = psa.tile([128, NT], F32, tag="acc",
                                        name="ps_o")
                        for j in range(MPC):
                            nc.tensor.matmul(
                                ps_o, wo_t[:, j, i * 128:(i + 1) * 128],
                                attn_m[j],
                                start=(j == 0), stop=(j == MPC - 1))
                        if i % 2 == 0:
                            nc.vector.tensor_copy(ost[:, 0, :], ps_o)
                        else:
                            nc.scalar.copy(ost[:, 1, :], ps_o)
                            nc.sync.dma_start(outP[:, tq, i // 2], ost)

                # ================= emission schedule ======================
                bb_stage1(0)
                phaseVT(0)
                bb_stage23(0)
                phaseVT(1)
                phaseVT(2)
                bb_stage45(0)
                phaseVT(3)
                for t in range(NQ):
                    nxt = t + 1 < NQ
                    if nxt:
                        bb_stage1(t + 1)
                        bb_stage23(t + 1)
                    am0 = phaseC_attn(t, 0)
                    am1 = phaseC_attn(t, 1)
                    if t == 0:
                        stage_trig(3)
                    if nxt:
                        bb_stage45(t + 1)
                    phaseC_out(t, [am0, am1])

    nc.compile()
    return nc


def _prep_inputs(hidden_states, cos, sin, Wq, Wk, Wv, Wo,
                 conv_q_w, conv_k_w, conv_v_w, q_norm_w, k_norm_w):
    f = np.float32
    bf = ml_dtypes.bfloat16
    x = np.asarray(hidden_states, f)[0]            # [T, D]
    xT = x.T.astype(bf)                            # [D, T] bf16
    # pack: xP[p, s, k, t'] = xT[k*128+p, s*512+t']
    xP = np.ascontiguousarray(
        xT.reshape(KD, 128, NQ, NT).transpose(1, 2, 0, 3))

    def packw(W, sl):
        # wP[p, k, c] = W.T[k*128+p, sl][c]
        WT = np.asarray(W, f).T[:, sl].astype(bf)  # [D, CPC]
        return np.ascontiguousarray(
            WT.reshape(KD, 128, CPC).transpose(1, 0, 2))

    cosT = np.asarray(cos, f)[0].T                 # [RD, T]
    sinT = np.asarray(sin, f)[0].T
    trig = np.zeros((128, T), f)
    trig[0:RD] = cosT
    # ssin2 block (device rows 0:64): [0:32] = +sin[32:64], [32:64] = -sin[0:32]
    trig[RD:RD + 32] = sinT[32:64]
    trig[RD + 32:2 * RD] = -sinT[0:32]

    # causal mask strip: mask[kl, c] = 1.0 iff kl <= c - 384
    kl = np.arange(128, dtype=f)[:, None]
    cc = np.arange(896, dtype=f)[None, :]
    maskb = (kl <= cc - 384).astype(bf)

    nw = np.zeros((128, 2), f)
    nw[:, 0] = np.asarray(q_norm_w, f)
    nw[:, 1] = np.asarray(k_norm_w, f)

    WoTf = np.asarray(Wo, f).T.astype(bf)          # [D rows=c, D cols=dout]

    in_maps = []
    for c in range(NCORES):
        sl = slice(c * CPC, (c + 1) * CPC)
        # conv diagonal stationaries [128, (pi, m, j), 128]
        convd = np.zeros((128, 3 * MPC * KCONV, 128), bf)
        idx = np.arange(128)
        for pi, cw in enumerate((conv_q_w, conv_k_w, conv_v_w)):
            w = np.asarray(cw, f)[sl]              # [CPC, K]
            for m in range(MPC):
                for j in range(KCONV):
                    convd[idx, (pi * MPC + m) * KCONV + j, idx] = \
                        w[m * 128 + idx, j].astype(bf)
        woP = np.ascontiguousarray(
            WoTf[sl].reshape(MPC, 128, D).transpose(1, 0, 2))
        in_maps.append({
            "xP": xP,
            "wqP": packw(Wq, sl),
            "wkP": packw(Wk, sl),
            "wvP": packw(Wv, sl),
            "woP": woP,
            "trig": trig,
            "convd": np.ascontiguousarray(convd),
            "normw": nw,
            "maskb": maskb,
        })
    return in_maps


def kernel(hidden_states, cos, sin, Wq, Wk, Wv, Wo,
           conv_q_w, conv_k_w, conv_v_w, q_norm_w, k_norm_w,
           _trace=False):
    global _COMPILED
    if _COMPILED is None:
        _COMPILED = _build()
    nc = _COMPILED
    in_maps = _prep_inputs(hidden_states, cos, sin, Wq, Wk, Wv, Wo,
                           conv_q_w, conv_k_w, conv_v_w, q_norm_w, k_norm_w)
    res = bass_utils.run_bass_kernel_spmd(
        nc, in_maps, core_ids=list(range(NCORES)), trace=_trace)
    acc = np.zeros((D, T), np.float64)
    for r in res.results:
        arr = np.asarray(r["outP"], dtype=np.float32)  # [128,tq,blk2,i,t]
        # rows (blk2*2+i)*128+p, cols tq*512+t
        acc += arr.transpose(2, 3, 0, 1, 4).reshape(D, T)
    out = np.ascontiguousarray(acc.T.astype(np.float32))[None]
    if _trace:
        kernel._last_results = res
    return out


# revision 21
# speedup vs baseline: 1.4216x; 1.2390x over previous
"""Trainium2 Bass kernel for nn_Attention_34033320854122.

Dense transformer attention block: QKV proj -> causal depthwise conv+SiLU ->
per-head RMSNorm -> partial RoPE -> causal attention -> output projection.

Sharding: tensor-parallel over the 16 heads across 8 NeuronCores (2 heads =
256 channels per core). Each core computes q/k/v for its channels (full
contraction over D), runs attention for its 2 heads, and produces a partial
output projection (outT_partial = Wo[:, cols] @ attn_cols^T, bf16). The host
sums the 8 partials and transposes.

Numerics / fidelity notes (same conventions as the validated baseline):
- The reference negates the rotated RoPE sub-dim of BOTH q and k; the
  negation cancels in q . k and is skipped.
- softmax runs without max-subtraction (scores bounded well inside fp32 exp
  range for RMS-normed q/k at scale 1/sqrt(128)).
- Matmuls: QKV/Wo projections and attention PV in bf16; QK in float32r.
- The depthwise conv runs on the PE as 4 accumulating matmuls with
  diagonal stationary matrices diag(w[:, j]) built on the host.
- RMSNorm rescale commutes past RoPE (per-position scalar), so the trig
  products are computed on the SiLU output early (hiding on the otherwise
  idle Vector engine during the projection phase) and the rstd * norm_w
  scale is applied at the end via two scalar_tensor_tensor ops.
- Activation table sets: silu wave first, then only {square, sqrt} around
  {exp}, with sqrt batches scheduled at q-tile starts (before exps are
  ready) to avoid per-op table-load thrash.
- All DRAM tensors are packed host-side so each DMA is one contiguous run
  per partition (HWDGE issue time scales with descriptor count).
"""

import ml_dtypes
import numpy as np

import concourse.bacc as bacc
import concourse.tile as tile
import concourse.mybir as mybir
from concourse import bass_utils
from concourse.masks import make_identity

# Problem shape (hardcoded per contract)
B, T, D = 1, 2048, 2048
H, HD = 16, 128
RD = 64
KCONV = 4
EPS = 1e-5
NCORES = 8
CPC = D // NCORES      # channels per core = 256
MPC = CPC // HD        # head tiles per core = 2
NT = 512               # free-dim tile for matmuls
NQ = T // NT           # 4 q tiles
NKC = T // HD          # 16 key chunks of 128
KD = D // 128          # 16 contraction chunks
PAD = KCONV - 1        # causal conv history

F32 = mybir.dt.float32
F32R = mybir.dt.float32r
BF16 = mybir.dt.bfloat16

_COMPILED = None


def _build():
    nc = bacc.Bacc("TRN2", target_bir_lowering=False, debug=False,
                   num_devices=NCORES)

    d = {}
    # packed layouts: leading dim 128 = SBUF partition
    d["xP"] = nc.dram_tensor("xP", (128, NQ, KD, NT), BF16,
                             kind="ExternalInput").ap()
    d["wqP"] = nc.dram_tensor("wqP", (128, KD, CPC), BF16,
                              kind="ExternalInput").ap()
    d["wkP"] = nc.dram_tensor("wkP", (128, KD, CPC), BF16,
                              kind="ExternalInput").ap()
    d["wvP"] = nc.dram_tensor("wvP", (128, KD, CPC), BF16,
                              kind="ExternalInput").ap()
    d["woP"] = nc.dram_tensor("woP", (128, MPC, D), BF16,
                              kind="ExternalInput").ap()
    # trig: rows 0:64 = cos^T, rows 64:128 = sign-folded sin^T
    d["trig"] = nc.dram_tensor("trig", (128, T), F32, kind="ExternalInput").ap()
    # conv weights as diagonal stationaries [128, (pi,m,j)=24, 128]
    d["convd"] = nc.dram_tensor("convd", (128, 3 * MPC * KCONV, 128), BF16,
                                kind="ExternalInput").ap()
    # per-head norm weights [128, 2] (q, k)
    d["normw"] = nc.dram_tensor("normw", (128, 2), F32, kind="ExternalInput").ap()
    # causal mask strip [128, 896]: mask[kl, c] = 1.0 if kl <= c - 384
    d["maskb"] = nc.dram_tensor("maskb", (128, 896), BF16,
                                kind="ExternalInput").ap()
    # packed output: [p, tq, blk2, i, t'] -> out row (blk2*2+i)*128+p
    outP = nc.dram_tensor("outP", (128, NQ, 8, 2, NT), BF16,
                          kind="ExternalOutput").ap()

    inv_sqrt_hd = 1.0 / np.sqrt(HD)

    with tile.TileContext(nc) as tc:
        with (
            tc.tile_pool(name="consts", bufs=1) as consts,
            tc.tile_pool(name="raw", bufs=1) as rawp,
            tc.tile_pool(name="wor", bufs=1) as worp,
            tc.tile_pool(name="final", bufs=1) as finalp,
            tc.tile_pool(name="vsil", bufs=8) as vsilp,
            tc.tile_pool(name="rc", bufs=3) as rcp,
            tc.tile_pool(name="scratch", bufs=2) as scr,
            tc.tile_pool(name="psa", bufs=3, space="PSUM") as psa,
            tc.tile_pool(name="psb", bufs=4, space="PSUM") as psb,
            tc.tile_pool(name="psone", bufs=1, space="PSUM") as psone,
        ):
            # ---- persistent buffers ----
            rawq = rawp.tile([128, MPC, T + PAD], BF16)
            rawk = rawp.tile([128, MPC, T + PAD], BF16)
            rawv = rawp.tile([128, MPC, T + PAD], BF16)
            qfT = finalp.tile([128, MPC, T], F32R)
            kfT = finalp.tile([128, MPC, T], F32R)
            vtr = finalp.tile([128, MPC, NKC, HD], BF16)
            raws = (rawq, rawk, rawv)
            groups = [[(0, 0), (0, 1), (1, 0)], [(1, 1), (2, 0), (2, 1)]]

            convd_t = consts.tile([128, 3 * MPC * KCONV, 128], BF16)
            cosT_t = consts.tile([64, T], F32)
            ssin2_t = consts.tile([64, T], F32)
            normw_t = consts.tile([128, 2], F32)
            mask_t = consts.tile([128, 896], BF16)
            wo_t = worp.tile([128, MPC, D], BF16)

            ones_f = consts.tile([128, 1], F32)
            nc.vector.memset(ones_f, 1.0)
            ones_bf = consts.tile([128, 1], BF16)   # lhsT for bf16 col sums
            nc.vector.tensor_copy(ones_bf, ones_f)
            ones_1f = consts.tile([1, 128], F32)
            nc.vector.memset(ones_1f, 1.0)
            ones_1 = consts.tile([1, 128], F32R)    # lhsT for bcast over parts
            nc.vector.tensor_copy(ones_1, ones_1f)
            eps_t = consts.tile([1, 1], F32)
            nc.vector.memset(eps_t, EPS)
            ident_f = consts.tile([128, 128], F32)
            make_identity(nc, ident_f)
            ident_bf = consts.tile([128, 128], BF16)
            nc.vector.tensor_copy(ident_bf, ident_f)
            for r in (rawq, rawk, rawv):
                nc.vector.memset(r[:, :, 0:PAD], 0.0)

            # ====== rope trig part on the silu output (rstd-independent):
            # rc[(s,m,pi)] rows 0:RD = qs*cos + rotate_half(qs)*ssin2
            def stage_trig(s):
                sl = slice(s * NT, (s + 1) * NT)
                for m in range(MPC):
                    for pi in range(2):
                        qs = raws[pi][:, m, s * NT:s * NT + NT]
                        rot2 = scr.tile([64, 2, NT], F32, tag="rot2")
                        nc.gpsimd.tensor_mul(rot2[0:32, 1, :], qs[32:64],
                                             ssin2_t[32:64, sl])
                        nc.gpsimd.tensor_mul(rot2[32:64, 1, :], qs[0:32],
                                             ssin2_t[0:32, sl])
                        nc.vector.tensor_mul(rot2[:, 0, :], qs[0:RD],
                                             cosT_t[:, sl])
                        rc = rcp.tile([64, NT], F32, tag=f"rc{m}{pi}",
                                      name="rc")
                        nc.vector.tensor_add(rc, rot2[:, 0, :],
                                             rot2[:, 1, :])
                        stage_trig.rc[(s, m, pi)] = rc
            stage_trig.rc = {}

            # =============== Phase A: QKV projection for slice s ==========
            def phaseA(s, w_all, xp, xb0=None):
                if s == 0:
                    xb = xb0
                else:
                    xb = xp.tile([128, KD, NT], BF16, name="xb")
                    deng = nc.sync if s % 2 == 0 else nc.scalar
                    deng.dma_start(xb, d["xP"][:, s])
                for grp in groups:
                    pst = [psa.tile([128, NT], F32, tag="acc",
                                    name=f"acc{gi}")
                           for gi in range(3)]
                    for k in range(KD):
                        for gi, (pi, m) in enumerate(grp):
                            nc.tensor.matmul(
                                pst[gi],
                                w_all[:, pi, k, m * 128:(m + 1) * 128],
                                xb[:, k, :],
                                start=(k == 0),
                                stop=(k == KD - 1),
                            )
                    for gi, (pi, m) in enumerate(grp):
                        dst = raws[pi][:, m,
                                       PAD + s * NT:PAD + (s + 1) * NT]
                        nc.scalar.copy(dst, pst[gi])

            # ====== Phase Ba: conv (PE diag matmuls) + SiLU for slice s ===
            def phaseBa(s):
                for m in range(MPC):
                    for pi in range(3):
                        cv = psb.tile([128, NT], F32, tag="sm", name="cv")
                        for j in range(KCONV):
                            nc.tensor.matmul(
                                cv,
                                convd_t[:, (pi * MPC + m) * KCONV + j, :],
                                raws[pi][:, m, s * NT + j:s * NT + j + NT],
                                start=(j == 0), stop=(j == KCONV - 1),
                            )
                        if pi < 2:
                            # silu back into raw, shifted 3 cols left
                            nc.scalar.activation(
                                raws[pi][:, m, s * NT:s * NT + NT], cv,
                                mybir.ActivationFunctionType.Silu)
                        else:
                            vv = vsilp.tile([128, NT], BF16, name="vv")
                            nc.scalar.activation(
                                vv, cv, mybir.ActivationFunctionType.Silu)
                            phaseBa.vv[(s, m)] = vv
            phaseBa.vv = {}

            # ====== vT(s): transpose v slice into [t, HD] layout ==========
            def phaseVT(s):
                for m in range(MPC):
                    vv = phaseBa.vv.pop((s, m))
                    ps_tr = psb.tile([128, NT], BF16, tag="sm", name="ps_tr")
                    for sub in range(NT // 128):
                        nc.tensor.transpose(
                            ps_tr[:, sub * 128:(sub + 1) * 128],
                            vv[:, sub * 128:(sub + 1) * 128], ident_bf)
                    nc.scalar.copy(
                        vtr[:, m, s * (NT // 128):(s + 1) * (NT // 128), :],
                        ps_tr.rearrange("p (s h) -> p s h", h=128))

            # ========== emission: projection + conv + silu + trig =========
            with (
                tc.tile_pool(name="wqkv", bufs=1) as wqkvp,
                tc.tile_pool(name="xblk", bufs=2) as xp,
            ):
                w_all = wqkvp.tile([128, 3, KD, CPC], BF16)
                xb0 = xp.tile([128, KD, NT], BF16, name="xb")
                nc.sync.dma_start(xb0[:, 0:4, :], d["xP"][:, 0, 0:4, :])
                wds = (d["wqP"], d["wkP"], d["wvP"])
                for kq in range(4):
                    for pi in range(3):
                        deng = nc.sync if (kq * 3 + pi) % 2 == 0 \
                            else nc.scalar
                        deng.dma_start(
                            w_all[:, pi, kq * 4:(kq + 1) * 4, :],
                            wds[pi][:, kq * 4:(kq + 1) * 4, :])
                    if kq < 3:
                        deng = nc.scalar if kq % 2 == 0 else nc.sync
                        deng.dma_start(
                            xb0[:, 4 * (kq + 1):4 * (kq + 2), :],
                            d["xP"][:, 0, 4 * (kq + 1):4 * (kq + 2), :])

                phaseA(0, w_all, xp, xb0)
                nc.gpsimd.dma_start(convd_t, d["convd"])
                phaseA(1, w_all, xp)
                phaseBa(0)
                nc.gpsimd.dma_start(cosT_t, d["trig"][0:64])
                nc.gpsimd.dma_start(ssin2_t, d["trig"][64:128])
                nc.gpsimd.dma_start(normw_t, d["normw"])
                phaseA(2, w_all, xp)
                stage_trig(0)
                phaseBa(1)
                nc.gpsimd.dma_start(mask_t, d["maskb"])
                phaseA(3, w_all, xp)
                stage_trig(1)
                phaseBa(2)
                nc.gpsimd.dma_start(wo_t, d["woP"])
                phaseBa(3)
                stage_trig(2)

            # ====== Phase Bb: RMS norm, staged for pipelining =============
            # stage1: squares (scalar).  stage23: partition-sum MM + sqrt +
            # recip + f32r round.  stage45: bcast MM + 2 STTs into qfT/kfT.
            with (
                tc.tile_pool(name="sqp", bufs=1) as sqp,
                tc.tile_pool(name="rsd", bufs=1) as rsdp,
                tc.tile_pool(name="exp", bufs=4) as expp,
                tc.tile_pool(name="attn", bufs=3) as attnp,
                tc.tile_pool(name="ostage", bufs=2) as ostp,
            ):
                bbst = {}

                def bb_stage1(s):
                    sq4 = sqp.tile([128, 4, NT], BF16, name="sq4")
                    for m in range(MPC):
                        for pi in range(2):
                            qs = raws[pi][:, m, s * NT:s * NT + NT]
                            nc.scalar.activation(
                                sq4[:, m * 2 + pi, :], qs,
                                mybir.ActivationFunctionType.Square)
                    bbst[s] = [sq4]

                def bb_stage23(s):
                    sq4 = bbst[s][0]
                    rsd4 = rsdp.tile([1, 4, NT], F32, name="rsd4")
                    rr4f = rsdp.tile([1, 4, NT], F32, name="rr4f")
                    rr4 = rsdp.tile([1, 4, NT], F32R, name="rr4")
                    for k in range(4):
                        ps_ss = psb.tile([1, NT], F32, tag="sm",
                                         name="ps_ss")
                        nc.tensor.matmul(ps_ss, ones_bf, sq4[:, k, :],
                                         start=True, stop=True)
                        nc.scalar.activation(
                            rsd4[:, k, :], ps_ss,
                            mybir.ActivationFunctionType.Sqrt,
                            scale=1.0 / HD, bias=eps_t)
                        nc.vector.reciprocal_approx_fast(
                            rr4f[:, k, :], rsd4[:, k, :])
                        nc.vector.tensor_copy(rr4[:, k, :], rr4f[:, k, :])
                    bbst[s].append(rr4)

                def bb_stage45(s):
                    sl = slice(s * NT, (s + 1) * NT)
                    rr4 = bbst.pop(s)[1]
                    for m in range(MPC):
                        for pi in range(2):
                            fin = qfT if pi == 0 else kfT
                            qs = raws[pi][:, m, s * NT:s * NT + NT]
                            rc = stage_trig.rc.pop((s, m, pi))
                            ps_rb = psb.tile([128, NT], F32, tag="sm",
                                             name="ps_rb")
                            nc.tensor.matmul(ps_rb, ones_1,
                                             rr4[:, m * 2 + pi, :],
                                             start=True, stop=True)
                            nc.vector.scalar_tensor_tensor(
                                fin[0:RD, m, sl], rc,
                                normw_t[0:RD, pi:pi + 1], ps_rb[0:RD, :],
                                mybir.AluOpType.mult, mybir.AluOpType.mult,
                            )
                            nc.vector.scalar_tensor_tensor(
                                fin[RD:128, m, sl], qs[RD:128],
                                normw_t[RD:128, pi:pi + 1],
                                ps_rb[RD:128, :],
                                mybir.AluOpType.mult, mybir.AluOpType.mult,
                            )

                # =========== Phase C: attention + output proj =============
                def phaseC_attn(tq, m):
                    qsl = slice(tq * NT, (tq + 1) * NT)
                    nch = 4 * tq + 4
                    ps_attn = psa.tile([128, NT], F32, tag="acc",
                                       name="ps_attn")
                    ps_sum = psone.tile([1, NT], F32, tag="one",
                                        name="ps_sum")

                    def qk(tk):
                        ps_s = psb.tile([128, NT], F32, tag="sm",
                                        name="ps_s")
                        nc.tensor.matmul(
                            ps_s, kfT[:, m, tk * 128:(tk + 1) * 128],
                            qfT[:, m, qsl], start=True, stop=True)
                        e = expp.tile([128, NT], BF16, tag="e", name="e")
                        nc.scalar.activation(
                            e, ps_s, mybir.ActivationFunctionType.Exp,
                            scale=inv_sqrt_hd)
                        dd = tk * 128 - tq * NT
                        if dd >= 0:  # diagonal chunk: causal mask
                            nc.vector.tensor_mul(
                                e, e, mask_t[:, 384 - dd:896 - dd])
                        return e

                    epipe = [qk(t) for t in range(min(3, nch))]
                    hold = None
                    for tk in range(nch):
                        if tk + 3 < nch:
                            epipe.append(qk(tk + 3))
                        e = epipe.pop(0)
                        nc.tensor.matmul(ps_attn, vtr[:, m, tk, :], e,
                                         start=(tk == 0),
                                         stop=(tk == nch - 1))
                        if tk % 2 == 0:
                            hold = e
                        else:
                            es = expp.tile([128, NT], BF16, tag="es",
                                           name="es")
                            nc.vector.tensor_add(es, hold, e)
                            nc.tensor.matmul(ps_sum, ones_bf, es,
                                             start=(tk == 1),
                                             stop=(tk == nch - 1))
                    # normalize: attn^T *= 1/sumexp (broadcast over parts)
                    rr = scr.tile([1, NT], F32, tag="rst", name="rrs")
                    nc.vector.reciprocal_approx_fast(rr, ps_sum)
                    rr_r = scr.tile([1, NT], F32R, tag="rsr", name="rrs_r")
                    nc.vector.tensor_copy(rr_r, rr)
                    ps_rb = psb.tile([128, NT], F32, tag="sm", name="ps_rb")
                    nc.tensor.matmul(ps_rb, ones_1, rr_r, start=True,
                                     stop=True)
                    rb = scr.tile([128, NT], F32, tag="rbs")
                    nc.scalar.copy(rb, ps_rb)
                    am = attnp.tile([128, NT], BF16, tag="am", name="am")
                    nc.vector.tensor_mul(am, ps_attn, rb)
                    return am

                def phaseC_out(tq, attn_m):
                    ost = None
                    for i in range(D // 128):
                        if i % 2 == 0:
                            ost = ostp.tile([128, 2, NT], BF16, name="ost")
                        ps_o = psa.tile([128, NT], F32, tag="acc",
                                        name="ps_o")
                        for j in range(MPC):
                            nc.tensor.matmul(
                                ps_o, wo_t[:, j, i * 128:(i + 1) * 128],
                                attn_m[j],
                                start=(j == 0), stop=(j == MPC - 1))
                        if i % 2 == 0:
                            nc.vector.tensor_copy(ost[:, 0, :], ps_o)
                        else:
                            nc.scalar.copy(ost[:, 1, :], ps_o)
                            deng = nc.sync if (i // 2) % 2 == 0 \
                                else nc.scalar
                            deng.dma_start(outP[:, tq, i // 2], ost)

                # ================= emission schedule ======================
                bb_stage1(0)
                phaseVT(0)
                bb_stage23(0)
                phaseVT(1)
                phaseVT(2)
                bb_stage45(0)
                phaseVT(3)
                for t in range(NQ):
                    nxt = t + 1 < NQ
                    if nxt:
                        bb_stage1(t + 1)
                        bb_stage23(t + 1)
                    am0 = phaseC_attn(t, 0)
                    am1 = phaseC_attn(t, 1)
                    if t == 0:
                        stage_trig(3)
                    if nxt:
                        bb_stage45(t + 1)
                    phaseC_out(t, [am0, am1])

    nc.compile()
    return nc


def _prep_inputs(hidden_states, cos, sin, Wq, Wk, Wv, Wo,
                 conv_q_w, conv_k_w, conv_v_w, q_norm_w, k_norm_w):
    f = np.float32
    bf = ml_dtypes.bfloat16
    x = np.asarray(hidden_states, f)[0]            # [T, D]
    xT = x.T.astype(bf)                            # [D, T] bf16
    # pack: xP[p, s, k, t'] = xT[k*128+p, s*512+t']
    xP = np.ascontiguousarray(
        xT.reshape(KD, 128, NQ, NT).transpose(1, 2, 0, 3))

    def packw(W, sl):
        # wP[p, k, c] = W.T[k*128+p, sl][c]
        WT = np.asarray(W, f).T[:, sl].astype(bf)  # [D, CPC]
        return np.ascontiguousarray(
            WT.reshape(KD, 128, CPC).transpose(1, 0, 2))

    cosT = np.asarray(cos, f)[0].T                 # [RD, T]
    sinT = np.asarray(sin, f)[0].T
    trig = np.zeros((128, T), f)
    trig[0:RD] = cosT
    # ssin2 block (device rows 0:64): [0:32] = +sin[32:64], [32:64] = -sin[0:32]
    trig[RD:RD + 32] = sinT[32:64]
    trig[RD + 32:2 * RD] = -sinT[0:32]

    # causal mask strip: mask[kl, c] = 1.0 iff kl <= c - 384
    kl = np.arange(128, dtype=f)[:, None]
    cc = np.arange(896, dtype=f)[None, :]
    maskb = (kl <= cc - 384).astype(bf)

    nw = np.zeros((128, 2), f)
    nw[:, 0] = np.asarray(q_norm_w, f)
    nw[:, 1] = np.asarray(k_norm_w, f)

    WoTf = np.asarray(Wo, f).T.astype(bf)          # [D rows=c, D cols=dout]

    in_maps = []
    for c in range(NCORES):
        sl = slice(c * CPC, (c + 1) * CPC)
        # conv diagonal stationaries [128, (pi, m, j), 128]
        convd = np.zeros((128, 3 * MPC * KCONV, 128), bf)
        idx = np.arange(128)
        for pi, cw in enumerate((conv_q_w, conv_k_w, conv_v_w)):
            w = np.asarray(cw, f)[sl]              # [CPC, K]
            for m in range(MPC):
                for j in range(KCONV):
                    convd[idx, (pi * MPC + m) * KCONV + j, idx] = \
                        w[m * 128 + idx, j].astype(bf)
        woP = np.ascontiguousarray(
            WoTf[sl].reshape(MPC, 128, D).transpose(1, 0, 2))
        in_maps.append({
            "xP": xP,
            "wqP": packw(Wq, sl),
            "wkP": packw(Wk, sl),
            "wvP": packw(Wv, sl),
            "woP": woP,
            "trig": trig,
            "convd": np.ascontiguousarray(convd),
            "normw": nw,
            "maskb": maskb,
        })
    return in_maps


def kernel(hidden_states, cos, sin, Wq, Wk, Wv, Wo,
           conv_q_w, conv_k_w, conv_v_w, q_norm_w, k_norm_w,
           _trace=False):
    global _COMPILED
    if _COMPILED is None:
        _COMPILED = _build()
    nc = _COMPILED
    in_maps = _prep_inputs(hidden_states, cos, sin, Wq, Wk, Wv, Wo,
                           conv_q_w, conv_k_w, conv_v_w, q_norm_w, k_norm_w)
    res = bass_utils.run_bass_kernel_spmd(
        nc, in_maps, core_ids=list(range(NCORES)), trace=_trace)
    acc = np.zeros((D, T), np.float64)
    for r in res.results:
        arr = np.asarray(r["outP"], dtype=np.float32)  # [128,tq,blk2,i,t]
        # rows (blk2*2+i)*128+p, cols tq*512+t
        acc += arr.transpose(2, 3, 0, 1, 4).reshape(D, T)
    out = np.ascontiguousarray(acc.T.astype(np.float32))[None]
    if _trace:
        kernel._last_results = res
    return out
